# revision 13
# baseline (speedup 1.0000x reference)
"""Trainium2 Bass kernel for nn_DeepUDI (RGAT+GRU message passing), 8-core SPMD.

Sharding: nodes (dim 0) split across 8 cores; 256 nodes = 512 (node,relation)
pairs per core. Neighbor gather + weight folding on host (graph-parallel, no
collectives), all per-pair math on device.

Host-side algebraic folds (exact in fp32, weights-only):
  A    = w @ kw @ qw^T @ w^T        (attention scores = hn . (A h))
  WnW0 = w @ Wn0,  WnW1 = w @ Wn1   (gate pre-acts act on g = hn^T softmax(E))

Device structure (all-fp16 operands, fp32 PSUM accumulation):
Pairs are PSUM columns. Per-pair matvecs are packed so each TensorE
LDWEIGHTS+MATMUL covers 2 pairs (4 for the 32-row hn stationary): stationaries
stack two pairs' matrices vertically in the 128 partitions, and the moving
operand holds block-"diagonal" columns ([v_e;0], [0;v_o]) so one matmul with
N=2 computes both pairs without cross-terms. This removes the fp32 2-pass
matmul penalty (fp32_mode=LOW_HIGH), halves score-path HBM bytes, and ~halves
the TensorE instruction count vs per-pair N=1 matmuls.

Stages per 128-pair tile:
  A : u = A h            lhsT=[A_e^T;A_o^T](128x64)       rhs=h-diag   N=2
  C : scores = hn u      lhsT=[hn_e^T;hn_o^T](128x32)     rhs=u-diag   N=2
  softmax over K=32 (exp on ACT, sums/broadcast via ones-matmuls)
  D : g = hn^T E         lhsT=[hn_0;..;hn_3](128x64)      rhs=E-diag   N=4
  RZ: [Rpre;Zpre]        lhsT=[[Wx0;WnW0]|[Wx1;WnW1]]     rhs=[h;g]    N=1
  W : df = w^T g         lhsT=[w_e;w_o](128x64)           rhs=g-diag   N=2
  H : Hpre               lhsT=[SH_e|SH_o](128x128)        rhs=[h;rdf]  N=2
      (SH=[Wx2;Wn2]; diag output rows 0:64 even cols / 64:128 odd cols)
  gru = Z df + (1-Z) tanh(Hpre+b2);  out = tanh(mean_r gru)
"""

import numpy as np

N, R, K, D, F = 2048, 2, 32, 64, 64
P_ALL = N * R            # 4096 pairs
NCORES = 8
PPC = P_ALL // NCORES    # 512 pairs/core
NPC = N // NCORES        # 256 nodes/core
TILE = 64                # pairs per tile
NT = PPC // TILE         # 8 tiles/core
U2 = TILE // 2           # 64 two-pair units
U4 = TILE // 4           # 32 four-pair units

_cache = {}


def _build():
    import concourse.mybir as mybir
    import concourse.tile as tile
    from concourse import bacc

    fp32 = mybir.dt.float32
    fp16 = mybir.dt.float16
    Sig = mybir.ActivationFunctionType.Sigmoid
    Tanh = mybir.ActivationFunctionType.Tanh
    Exp = mybir.ActivationFunctionType.Exp

    nc = bacc.Bacc(
        "TRN2", target_bir_lowering=False, debug=False, num_devices=NCORES
    )

    # ---- DRAM I/O (per-core shards) ----
    dSA = nc.dram_tensor("SA", [NT, 128, U2 * D], fp16, kind="ExternalInput")
    dSC = nc.dram_tensor("SC", [NT, 128, U2 * K], fp16, kind="ExternalInput")
    dSD = nc.dram_tensor("SD", [NT, 128, U4 * D], fp16, kind="ExternalInput")
    dSRZ = nc.dram_tensor("SRZ", [NT, 128, TILE * 2 * F], fp16, kind="ExternalInput")
    dSW = nc.dram_tensor("SW", [NT, 128, U2 * F], fp16, kind="ExternalInput")
    dSH = nc.dram_tensor("SH", [NT, 128, U2 * 2 * F], fp16, kind="ExternalInput")
    dHD = nc.dram_tensor("HD", [NT, 128, TILE], fp16, kind="ExternalInput")
    dHb = nc.dram_tensor("Hb", [NT, D, TILE], fp16, kind="ExternalInput")
    dBB = nc.dram_tensor("BB", [NT, 128, 2 * TILE], fp32, kind="ExternalInput")
    dOut = nc.dram_tensor("out", [NT, F, U2], fp32, kind="ExternalOutput")

    with tile.TileContext(nc) as tc:
        with (
            tc.tile_pool(name="const", bufs=1) as cpool,
            tc.tile_pool(name="stat", bufs=4) as spool,
            tc.tile_pool(name="vec", bufs=3) as vpool,
            tc.tile_pool(name="p64", bufs=4, space="PSUM") as p64_pool,
            tc.tile_pool(name="pM", bufs=2, space="PSUM") as pM_pool,
            tc.tile_pool(name="p128", bufs=2, space="PSUM") as p128_pool,
        ):
            ones_k1 = cpool.tile([K, 1], fp32)
            nc.vector.memset(ones_k1, 1.0)
            ones_1k = cpool.tile([1, K], fp32)
            nc.vector.memset(ones_1k, 1.0)

            for t in range(NT):
                # ---- stationaries + vectors for this tile ----
                sa = spool.tile([128, U2 * D], fp16, tag="sa")
                nc.sync.dma_start(out=sa, in_=dSA[t])
                sc = spool.tile([128, U2 * K], fp16, tag="sc")
                nc.sync.dma_start(out=sc, in_=dSC[t])
                sd = spool.tile([128, U4 * D], fp16, tag="sd")
                nc.sync.dma_start(out=sd, in_=dSD[t])
                srz = spool.tile([128, TILE * 2 * F], fp16, tag="srz")
                nc.sync.dma_start(out=srz, in_=dSRZ[t])
                sw = spool.tile([128, U2 * F], fp16, tag="sw")
                nc.sync.dma_start(out=sw, in_=dSW[t])
                sh = spool.tile([128, U2 * 2 * F], fp16, tag="sh")
                nc.sync.dma_start(out=sh, in_=dSH[t])
                hd = vpool.tile([128, TILE], fp16, tag="hd")
                nc.sync.dma_start(out=hd, in_=dHD[t])
                hg = vpool.tile([128, TILE], fp16, tag="hg")
                nc.sync.dma_start(out=hg[0:D, :], in_=dHb[t])
                hrdf = vpool.tile([128, TILE], fp16, tag="hrdf")
                nc.sync.dma_start(out=hrdf[0:D, :], in_=dHb[t])
                bb = vpool.tile([128, 2 * TILE], fp32, tag="bb")
                nc.sync.dma_start(out=bb, in_=dBB[t])

                # ---- stage A: u = A h ----
                pA = p64_pool.tile([D, TILE], fp32, tag="p64")
                for i in range(U2):
                    nc.tensor.matmul(
                        pA[:, 2 * i : 2 * i + 2],
                        sa[:, i * D : (i + 1) * D],
                        hd[:, 2 * i : 2 * i + 2],
                        start=True, stop=True,
                    )
                # u-diag restack: col 2i = [u_e;0], col 2i+1 = [0;u_o]
                uc = vpool.tile([128, TILE], fp16, tag="uc")
                nc.vector.memset(uc, 0.0)
                pA_v = pA.rearrange("d (u r) -> d r u", r=2)
                nc.vector.tensor_copy(
                    uc[0:D, :].rearrange("d (u r) -> d r u", r=2)[:, 0, :],
                    pA_v[:, 0, :],
                )
                nc.vector.tensor_copy(
                    uc[D:128, :].rearrange("d (u r) -> d r u", r=2)[:, 1, :],
                    pA_v[:, 1, :],
                )

                # ---- stage C: scores = hn u ----
                pM = pM_pool.tile([K, 3 * TILE], fp32, tag="pM")
                psc = pM[:, 0:TILE]
                for i in range(U2):
                    nc.tensor.matmul(
                        psc[:, 2 * i : 2 * i + 2],
                        sc[:, i * K : (i + 1) * K],
                        uc[:, 2 * i : 2 * i + 2],
                        start=True, stop=True,
                    )
                # softmax over K: E = exp(scores); Ehat = E / sum_k E
                E_sb = vpool.tile([K, TILE], fp32, tag="esb")
                nc.scalar.activation(out=E_sb, in_=psc, func=Exp)
                ps_row = pM[0:1, TILE : 2 * TILE]
                nc.tensor.matmul(ps_row, ones_k1, E_sb, start=True, stop=True)
                rs_sb = vpool.tile([1, TILE], fp32, tag="rssb")
                nc.vector.reciprocal(rs_sb, ps_row)
                prsb = pM[:, 2 * TILE : 3 * TILE]
                nc.tensor.matmul(prsb, ones_1k, rs_sb, start=True, stop=True)
                ehat = vpool.tile([K, TILE], fp32, tag="ehat")
                nc.vector.tensor_tensor(
                    out=ehat, in0=E_sb, in1=prsb, op=mybir.AluOpType.mult
                )
                # E-diag restack: col 4i+j has E at rows 32j:32j+32
                ed = vpool.tile([128, TILE], fp16, tag="ed")
                nc.vector.memset(ed, 0.0)
                ehat_v = ehat.rearrange("k (u r) -> k r u", r=4)
                for j in range(4):
                    nc.vector.tensor_copy(
                        ed[32 * j : 32 * j + 32, :]
                        .rearrange("k (u r) -> k r u", r=4)[:, j, :],
                        ehat_v[:, j, :],
                    )

                # ---- stage D: g = hn^T Ehat ----
                pG = p64_pool.tile([D, TILE], fp32, tag="p64")
                for i in range(U4):
                    nc.tensor.matmul(
                        pG[:, 4 * i : 4 * i + 4],
                        sd[:, i * D : (i + 1) * D],
                        ed[:, 4 * i : 4 * i + 4],
                        start=True, stop=True,
                    )
                # hg rows D:128 = g (straight); g-diag restack for stage W
                nc.vector.tensor_copy(hg[D:128, :], pG)
                gd = vpool.tile([128, TILE], fp16, tag="gd")
                nc.vector.memset(gd, 0.0)
                pG_v = pG.rearrange("d (u r) -> d r u", r=2)
                nc.vector.tensor_copy(
                    gd[0:D, :].rearrange("d (u r) -> d r u", r=2)[:, 0, :],
                    pG_v[:, 0, :],
                )
                nc.vector.tensor_copy(
                    gd[D:128, :].rearrange("d (u r) -> d r u", r=2)[:, 1, :],
                    pG_v[:, 1, :],
                )

                # ---- stage RZ: [Rpre;Zpre] = [[Wx0;WnW0]|[Wx1;WnW1]]^T [h;g] ----
                pRZ = p128_pool.tile([128, TILE], fp32, tag="p128")
                for p in range(TILE):
                    nc.tensor.matmul(
                        pRZ[:, p : p + 1],
                        srz[:, p * 2 * F : (p + 1) * 2 * F],
                        hg[:, p : p + 1],
                        start=True, stop=True,
                    )
                RZp = vpool.tile([128, TILE], fp32, tag="RZp")
                nc.vector.tensor_add(RZp, pRZ, bb[:, 0:TILE])
                RZs = vpool.tile([128, TILE], fp32, tag="RZs")
                nc.scalar.activation(out=RZs, in_=RZp, func=Sig)
                Zg = vpool.tile([F, TILE], fp32, tag="Zg")
                nc.vector.tensor_copy(Zg, RZs[D:128, :])

                # ---- stage W: df = w^T g ----
                pDF = p64_pool.tile([F, TILE], fp32, tag="p64")
                for i in range(U2):
                    nc.tensor.matmul(
                        pDF[:, 2 * i : 2 * i + 2],
                        sw[:, i * F : (i + 1) * F],
                        gd[:, 2 * i : 2 * i + 2],
                        start=True, stop=True,
                    )
                # hrdf rows D:128 = sigmoid(Rpre) * df
                rdf = vpool.tile([F, TILE], fp32, tag="rdf")
                nc.vector.tensor_tensor(
                    out=rdf, in0=RZs[0:D, :], in1=pDF,
                    op=mybir.AluOpType.mult,
                )
                nc.vector.tensor_copy(hrdf[D:128, :], rdf)

                # ---- stage H: Hpre (diag out) ----
                pH = p128_pool.tile([128, TILE], fp32, tag="p128")
                for i in range(U2):
                    nc.tensor.matmul(
                        pH[:, 2 * i : 2 * i + 2],
                        sh[:, i * 2 * F : (i + 1) * 2 * F],
                        hrdf[:, 2 * i : 2 * i + 2],
                        start=True, stop=True,
                    )
                tHp = vpool.tile([128, TILE], fp32, tag="tHp")
                nc.vector.tensor_add(tHp, pH, bb[:, TILE : 2 * TILE])
                Hcd = vpool.tile([128, TILE], fp32, tag="Hcd")
                nc.scalar.activation(out=Hcd, in_=tHp, func=Tanh)
                # assemble Hc straight [F, TILE] from diag halves
                Hc = vpool.tile([F, TILE], fp32, tag="Hc")
                Hc_v = Hc.rearrange("f (u r) -> f r u", r=2)
                nc.vector.tensor_copy(
                    Hc_v[:, 0, :],
                    Hcd[0:D, :].rearrange("f (u r) -> f r u", r=2)[:, 0, :],
                )
                nc.vector.tensor_copy(
                    Hc_v[:, 1, :],
                    Hcd[D:128, :].rearrange("f (u r) -> f r u", r=2)[:, 1, :],
                )

                # ---- gru = Hc + Z*(df - Hc); out = tanh(mean_r gru) ----
                gru = vpool.tile([F, TILE], fp32, tag="gru")
                nc.vector.tensor_sub(gru, pDF, Hc)
                nc.vector.tensor_mul(gru, gru, Zg)
                nc.vector.tensor_add(gru, gru, Hc)
                tcol = vpool.tile([F, U2], fp32, tag="tcol")
                gru_v = gru.rearrange("f (u r) -> f r u", r=2)
                nc.vector.tensor_add(tcol, gru_v[:, 0, :], gru_v[:, 1, :])
                osb = vpool.tile([F, U2], fp32, tag="osb")
                nc.scalar.activation(out=osb, in_=tcol, func=Tanh, scale=0.5)
                nc.sync.dma_start(out=dOut[t], in_=osb)

    nc.compile()
    return nc


def _prep(inputs):
    f16 = np.float16
    x = np.asarray(inputs["x"]).astype(np.int64)
    nbr = np.asarray(inputs["neighbors"]).astype(np.int64).reshape(P_ALL, K)
    embed = np.asarray(inputs["embed"], dtype=np.float32)
    w = np.asarray(inputs["w"], dtype=np.float32).reshape(P_ALL, D, F)
    qw = np.asarray(inputs["qw"], dtype=np.float32).reshape(P_ALL, F, -1)
    kw = np.asarray(inputs["kw"], dtype=np.float32).reshape(P_ALL, F, -1)
    Wx = np.asarray(inputs["Wx"], dtype=np.float32).reshape(P_ALL, 3, D, F)
    Wn = np.asarray(inputs["Wn"], dtype=np.float32).reshape(P_ALL, 3, F, F)
    b = (
        np.asarray(inputs["bx"], dtype=np.float32)
        + np.asarray(inputs["bn"], dtype=np.float32)
    ).reshape(P_ALL, 3, F)

    h = embed[x]                                    # [N, D]
    hp = np.repeat(h, R, axis=0)                    # [P, D] center node per pair
    hn = h[nbr]                                     # [P, K, D]
    A = w @ kw @ qw.transpose(0, 2, 1) @ w.transpose(0, 2, 1)   # [P, D, D]
    WnW0 = w @ Wn[:, 0]
    WnW1 = w @ Wn[:, 1]

    def stack2(M):
        # [PPC, D, X] -> [NT, 128, U2*X]: rows r*64+d, cols u*X+j
        X = M.shape[2]
        return (
            M.reshape(NT, U2, 2, D, X)
            .transpose(0, 2, 3, 1, 4)
            .reshape(NT, 128, U2 * X)
        )

    in_maps = []
    for c in range(NCORES):
        s = slice(c * PPC, (c + 1) * PPC)
        A_c, hn_c, w_c = A[s], hn[s], w[s]
        Wx_c, Wn2_c = Wx[s], Wn[s, 2]
        W0_c, W1_c = WnW0[s], WnW1[s]
        b_c, hp_c = b[s], hp[s]

        SA = stack2(A_c.transpose(0, 2, 1))                     # A^T blocks
        SC = stack2(hn_c.transpose(0, 2, 1))                    # hn^T blocks
        SD = (
            hn_c.reshape(NT, U4, 4, K, D)
            .transpose(0, 2, 3, 1, 4)
            .reshape(NT, 128, U4 * D)
        )
        RZblk = np.concatenate(
            [
                np.concatenate([Wx_c[:, 0], W0_c], axis=1),   # [PPC, 128, F]
                np.concatenate([Wx_c[:, 1], W1_c], axis=1),
            ],
            axis=2,
        )                                                      # [PPC, 128, 2F]
        SRZ = (
            RZblk.reshape(NT, TILE, 128, 2 * F)
            .transpose(0, 2, 1, 3)
            .reshape(NT, 128, TILE * 2 * F)
        )
        SW = stack2(w_c)
        SHp = np.concatenate([Wx_c[:, 2], Wn2_c], axis=1)       # [PPC, 128, F]
        SH = (
            SHp.reshape(NT, U2, 2, 128, F)
            .transpose(0, 3, 1, 2, 4)
            .reshape(NT, 128, U2 * 2 * F)
        )
        hp_t = hp_c.reshape(NT, TILE, D)
        Z = np.zeros((NT, TILE, 128), np.float32)
        Z[:, 0::2, 0:D] = hp_t[:, 0::2]
        Z[:, 1::2, D:128] = hp_t[:, 1::2]
        HD = Z.transpose(0, 2, 1)
        Hb = hp_t.transpose(0, 2, 1)
        B01 = (
            np.concatenate([b_c[:, 0], b_c[:, 1]], axis=1)
            .reshape(NT, TILE, 128)
            .transpose(0, 2, 1)
        )
        b2t = b_c[:, 2].reshape(NT, TILE, F)
        Z2 = np.zeros((NT, TILE, 128), np.float32)
        Z2[:, 0::2, 0:F] = b2t[:, 0::2]
        Z2[:, 1::2, F:128] = b2t[:, 1::2]
        B2D = Z2.transpose(0, 2, 1)
        BB = np.concatenate([B01, B2D], axis=2)

        m = {
            "SA": np.ascontiguousarray(SA).astype(f16),
            "SC": np.ascontiguousarray(SC).astype(f16),
            "SD": np.ascontiguousarray(SD).astype(f16),
            "SRZ": np.ascontiguousarray(SRZ).astype(f16),
            "SW": np.ascontiguousarray(SW).astype(f16),
            "SH": np.ascontiguousarray(SH).astype(f16),
            "HD": np.ascontiguousarray(HD).astype(f16),
            "Hb": np.ascontiguousarray(Hb).astype(f16),
            "BB": np.ascontiguousarray(BB),
        }
        in_maps.append(m)
    return in_maps


def kernel(**inputs):
    from concourse.bass_utils import run_bass_kernel_spmd

    if "nc" not in _cache:
        _cache["nc"] = _build()
    in_maps = _prep(inputs)
    res = run_bass_kernel_spmd(_cache["nc"], in_maps, list(range(NCORES)))
    outs = []
    for c in range(NCORES):
        o = res.results[c]["out"]                   # [NT, F, U2]
        outs.append(o.transpose(0, 2, 1).reshape(NPC, F))
    return np.concatenate(outs, axis=0).astype(np.float32)


# revision 17
# speedup vs baseline: 1.3302x; 1.3302x over previous
"""Trainium2 Bass kernel for nn_DeepUDI (RGAT+GRU message passing), 8-core SPMD.

Sharding: nodes (dim 0) split across 8 cores; 256 nodes = 512 (node,relation)
pairs per core. Neighbor gather + weight folding on host (graph-parallel, no
collectives), all per-pair math on device.

Host-side algebraic folds (exact in fp32, weights-only):
  A    = w @ kw @ qw^T @ w^T        (attention scores = hn . (A h))
  WnW0 = w @ Wn0,  WnW1 = w @ Wn1   (gate pre-acts act on g = hn^T softmax(E))

Device structure (all-fp16 operands, fp32 PSUM accumulation):
Pairs are PSUM columns. Per-pair matvecs are packed so each TensorE
LDWEIGHTS+MATMUL covers 2 pairs (4 for the 32-row hn stationary): stationaries
stack two pairs' matrices vertically in the 128 partitions, and the moving
operand holds block-"diagonal" columns ([v_e;0], [0;v_o]) so one matmul with
N=2 computes both pairs without cross-terms. This removes the fp32 2-pass
matmul penalty (fp32_mode=LOW_HIGH), halves score-path HBM bytes, and ~halves
the TensorE instruction count vs per-pair N=1 matmuls.

Stages per 128-pair tile:
  A : u = A h            lhsT=[A_e^T;A_o^T](128x64)       rhs=h-diag   N=2
  C : scores = hn u      lhsT=[hn_e^T;hn_o^T](128x32)     rhs=u-diag   N=2
  softmax over K=32 (exp on ACT, sums/broadcast via ones-matmuls)
  D : g = hn^T E         lhsT=[hn_0;..;hn_3](128x64)      rhs=E-diag   N=4
  RZ: [Rpre;Zpre]        lhsT=[[Wx0;WnW0]|[Wx1;WnW1]]     rhs=[h;g]    N=1
  W : df = w^T g         lhsT=[w_e;w_o](128x64)           rhs=g-diag   N=2
  H : Hpre               lhsT=[SH_e|SH_o](128x128)        rhs=[h;rdf]  N=2
      (SH=[Wx2;Wn2]; diag output rows 0:64 even cols / 64:128 odd cols)
  gru = Z df + (1-Z) tanh(Hpre+b2);  out = tanh(mean_r gru)
"""

import numpy as np

N, R, K, D, F = 2048, 2, 32, 64, 64
P_ALL = N * R            # 4096 pairs
NCORES = 8
PPC = P_ALL // NCORES    # 512 pairs/core
NPC = N // NCORES        # 256 nodes/core
TILE = 128               # pairs per tile
NT = PPC // TILE         # 4 tiles/core
U2 = TILE // 2           # 64 two-pair units
U4 = TILE // 4           # 32 four-pair units

_cache = {}


def _build():
    import concourse.mybir as mybir
    import concourse.tile as tile
    from concourse import bacc

    fp32 = mybir.dt.float32
    fp16 = mybir.dt.float16
    Sig = mybir.ActivationFunctionType.Sigmoid
    Tanh = mybir.ActivationFunctionType.Tanh
    Exp = mybir.ActivationFunctionType.Exp

    nc = bacc.Bacc(
        "TRN2", target_bir_lowering=False, debug=False, num_devices=NCORES
    )

    # ---- DRAM I/O (per-core shards) ----
    dSA = nc.dram_tensor("SA", [NT, 128, U2 * D], fp16, kind="ExternalInput")
    dSC = nc.dram_tensor("SC", [NT, 128, U2 * K], fp16, kind="ExternalInput")
    dSD = nc.dram_tensor("SD", [NT, 128, U4 * D], fp16, kind="ExternalInput")
    dSRZ = nc.dram_tensor("SRZ", [NT, 128, TILE * 2 * F], fp16, kind="ExternalInput")
    dSW = nc.dram_tensor("SW", [NT, 128, U2 * F], fp16, kind="ExternalInput")
    dSH = nc.dram_tensor("SH", [NT, 128, U2 * 2 * F], fp16, kind="ExternalInput")
    dHD = nc.dram_tensor("HD", [NT, 128, TILE], fp16, kind="ExternalInput")
    dHb = nc.dram_tensor("Hb", [NT, D, TILE], fp16, kind="ExternalInput")
    dBB = nc.dram_tensor("BB", [NT, 128, 2 * TILE], fp32, kind="ExternalInput")
    dOut = nc.dram_tensor("out", [NT, F, U2], fp32, kind="ExternalOutput")

    with tile.TileContext(nc) as tc:
        with (
            tc.tile_pool(name="const", bufs=1) as cpool,
            tc.tile_pool(name="stat", bufs=2) as spool,
            tc.tile_pool(name="big", bufs=3) as bpool,
            tc.tile_pool(name="vec", bufs=2) as vpool,
            tc.tile_pool(name="pA", bufs=2, space="PSUM") as pA_pool,
            tc.tile_pool(name="pM", bufs=2, space="PSUM") as pM_pool,
            tc.tile_pool(name="pGDF", bufs=2, space="PSUM") as pGDF_pool,
            tc.tile_pool(name="pRZ", bufs=1, space="PSUM") as pRZ_pool,
            tc.tile_pool(name="pH", bufs=1, space="PSUM") as pH_pool,
        ):
            ones_k1 = cpool.tile([K, 1], fp32)
            nc.vector.memset(ones_k1, 1.0)
            ones_1k = cpool.tile([1, K], fp32)
            nc.vector.memset(ones_1k, 1.0)

            for t in range(NT):
                # ---- stationaries + vectors for this tile ----
                sa = spool.tile([128, U2 * D], fp16, tag="sa")
                nc.sync.dma_start(out=sa, in_=dSA[t])
                sc = spool.tile([128, U2 * K], fp16, tag="sc")
                nc.sync.dma_start(out=sc, in_=dSC[t])
                sd = spool.tile([128, U4 * D], fp16, tag="sd")
                nc.sync.dma_start(out=sd, in_=dSD[t])
                srz = bpool.tile([128, TILE * 2 * F], fp16, tag="srz")
                nc.sync.dma_start(out=srz, in_=dSRZ[t])
                sw = spool.tile([128, U2 * F], fp16, tag="sw")
                nc.sync.dma_start(out=sw, in_=dSW[t])
                sh = spool.tile([128, U2 * 2 * F], fp16, tag="sh")
                nc.sync.dma_start(out=sh, in_=dSH[t])
                hd = vpool.tile([128, TILE], fp16, tag="hd")
                nc.sync.dma_start(out=hd, in_=dHD[t])
                hg = vpool.tile([128, TILE], fp16, tag="hg")
                nc.sync.dma_start(out=hg[0:D, :], in_=dHb[t])
                hrdf = vpool.tile([128, TILE], fp16, tag="hrdf")
                nc.sync.dma_start(out=hrdf[0:D, :], in_=dHb[t])
                bb = vpool.tile([128, 2 * TILE], fp32, tag="bb")
                nc.sync.dma_start(out=bb, in_=dBB[t])

                # ---- stage A: u = A h ----
                pA = pA_pool.tile([D, TILE], fp32, tag="pA")
                for i in range(U2):
                    nc.tensor.matmul(
                        pA[:, 2 * i : 2 * i + 2],
                        sa[:, i * D : (i + 1) * D],
                        hd[:, 2 * i : 2 * i + 2],
                        start=True, stop=True,
                    )
                # u-diag restack: col 2i = [u_e;0], col 2i+1 = [0;u_o]
                uc = vpool.tile([128, TILE], fp16, tag="uc")
                nc.vector.memset(uc, 0.0)
                pA_v = pA.rearrange("d (u r) -> d r u", r=2)
                nc.vector.tensor_copy(
                    uc[0:D, :].rearrange("d (u r) -> d r u", r=2)[:, 0, :],
                    pA_v[:, 0, :],
                )
                nc.vector.tensor_copy(
                    uc[D:128, :].rearrange("d (u r) -> d r u", r=2)[:, 1, :],
                    pA_v[:, 1, :],
                )

                # ---- stage C: scores = hn u ----
                pM = pM_pool.tile([K, 3 * TILE], fp32, tag="pM")
                psc = pM[:, 0:TILE]
                for i in range(U2):
                    nc.tensor.matmul(
                        psc[:, 2 * i : 2 * i + 2],
                        sc[:, i * K : (i + 1) * K],
                        uc[:, 2 * i : 2 * i + 2],
                        start=True, stop=True,
                    )
                # softmax over K: E = exp(scores); Ehat = E / sum_k E
                E_sb = vpool.tile([K, TILE], fp32, tag="esb")
                nc.scalar.activation(out=E_sb, in_=psc, func=Exp)
                ps_row = pM[0:1, TILE : 2 * TILE]
                nc.tensor.matmul(ps_row, ones_k1, E_sb, start=True, stop=True)
                rs_sb = vpool.tile([1, TILE], fp32, tag="rssb")
                nc.vector.reciprocal(rs_sb, ps_row)
                prsb = pM[:, 2 * TILE : 3 * TILE]
                nc.tensor.matmul(prsb, ones_1k, rs_sb, start=True, stop=True)
                ehat = vpool.tile([K, TILE], fp32, tag="ehat")
                nc.vector.tensor_tensor(
                    out=ehat, in0=E_sb, in1=prsb, op=mybir.AluOpType.mult
                )
                # E-diag restack: col 4i+j has E at rows 32j:32j+32
                ed = vpool.tile([128, TILE], fp16, tag="ed")
                nc.vector.memset(ed, 0.0)
                ehat_v = ehat.rearrange("k (u r) -> k r u", r=4)
                for j in range(4):
                    nc.vector.tensor_copy(
                        ed[32 * j : 32 * j + 32, :]
                        .rearrange("k (u r) -> k r u", r=4)[:, j, :],
                        ehat_v[:, j, :],
                    )

                # ---- stage D: g = hn^T Ehat ----
                pG = pGDF_pool.tile([D, TILE], fp32, tag="pgdf")
                for i in range(U4):
                    nc.tensor.matmul(
                        pG[:, 4 * i : 4 * i + 4],
                        sd[:, i * D : (i + 1) * D],
                        ed[:, 4 * i : 4 * i + 4],
                        start=True, stop=True,
                    )
                # hg rows D:128 = g (straight); g-diag restack for stage W
                nc.vector.tensor_copy(hg[D:128, :], pG)
                gd = vpool.tile([128, TILE], fp16, tag="gd")
                nc.vector.memset(gd, 0.0)
                pG_v = pG.rearrange("d (u r) -> d r u", r=2)
                nc.vector.tensor_copy(
                    gd[0:D, :].rearrange("d (u r) -> d r u", r=2)[:, 0, :],
                    pG_v[:, 0, :],
                )
                nc.vector.tensor_copy(
                    gd[D:128, :].rearrange("d (u r) -> d r u", r=2)[:, 1, :],
                    pG_v[:, 1, :],
                )

                # ---- stage RZ: [Rpre;Zpre] = [[Wx0;WnW0]|[Wx1;WnW1]]^T [h;g] ----
                pRZ = pRZ_pool.tile([128, TILE], fp32, tag="pRZ")
                for p in range(TILE):
                    nc.tensor.matmul(
                        pRZ[:, p : p + 1],
                        srz[:, p * 2 * F : (p + 1) * 2 * F],
                        hg[:, p : p + 1],
                        start=True, stop=True,
                    )
                RZp = vpool.tile([128, TILE], fp32, tag="RZp")
                nc.vector.tensor_add(RZp, pRZ, bb[:, 0:TILE])
                RZt = vpool.tile([128, TILE], fp32, tag="RZt")
                nc.scalar.activation(out=RZt, in_=RZp, func=Tanh, scale=0.5)
                RZs = vpool.tile([128, TILE], fp32, tag="RZs")
                nc.vector.tensor_scalar(
                    out=RZs, in0=RZt, scalar1=0.5, scalar2=0.5,
                    op0=mybir.AluOpType.mult, op1=mybir.AluOpType.add,
                )
                Zg = vpool.tile([F, TILE], fp32, tag="Zg")
                nc.vector.tensor_copy(Zg, RZs[D:128, :])

                # ---- stage W: df = w^T g ----
                pDF = pGDF_pool.tile([F, TILE], fp32, tag="pgdf")
                for i in range(U2):
                    nc.tensor.matmul(
                        pDF[:, 2 * i : 2 * i + 2],
                        sw[:, i * F : (i + 1) * F],
                        gd[:, 2 * i : 2 * i + 2],
                        start=True, stop=True,
                    )
                # hrdf rows D:128 = sigmoid(Rpre) * df
                rdf = vpool.tile([F, TILE], fp32, tag="rdf")
                nc.vector.tensor_tensor(
                    out=rdf, in0=RZs[0:D, :], in1=pDF,
                    op=mybir.AluOpType.mult,
                )
                nc.vector.tensor_copy(hrdf[D:128, :], rdf)

                # ---- stage H: Hpre (diag out) ----
                pH = pH_pool.tile([128, TILE], fp32, tag="pH")
                for i in range(U2):
                    nc.tensor.matmul(
                        pH[:, 2 * i : 2 * i + 2],
                        sh[:, i * 2 * F : (i + 1) * 2 * F],
                        hrdf[:, 2 * i : 2 * i + 2],
                        start=True, stop=True,
                    )
                tHp = vpool.tile([128, TILE], fp32, tag="tHp")
                nc.vector.tensor_add(tHp, pH, bb[:, TILE : 2 * TILE])
                Hcd = vpool.tile([128, TILE], fp32, tag="Hcd")
                nc.scalar.activation(out=Hcd, in_=tHp, func=Tanh)
                # assemble Hc straight [F, TILE] from diag halves
                Hc = vpool.tile([F, TILE], fp32, tag="Hc")
                Hc_v = Hc.rearrange("f (u r) -> f r u", r=2)
                nc.vector.tensor_copy(
                    Hc_v[:, 0, :],
                    Hcd[0:D, :].rearrange("f (u r) -> f r u", r=2)[:, 0, :],
                )
                nc.vector.tensor_copy(
                    Hc_v[:, 1, :],
                    Hcd[D:128, :].rearrange("f (u r) -> f r u", r=2)[:, 1, :],
                )

                # ---- gru = Hc + Z*(df - Hc); out = tanh(mean_r gru) ----
                gru = vpool.tile([F, TILE], fp32, tag="gru")
                nc.vector.tensor_sub(gru, pDF, Hc)
                nc.vector.tensor_mul(gru, gru, Zg)
                nc.vector.tensor_add(gru, gru, Hc)
                tcol = vpool.tile([F, U2], fp32, tag="tcol")
                gru_v = gru.rearrange("f (u r) -> f r u", r=2)
                nc.vector.tensor_add(tcol, gru_v[:, 0, :], gru_v[:, 1, :])
                osb = vpool.tile([F, U2], fp32, tag="osb")
                nc.scalar.activation(out=osb, in_=tcol, func=Tanh, scale=0.5)
                nc.sync.dma_start(out=dOut[t], in_=osb)

    nc.compile()
    return nc


def _prep(inputs):
    f16 = np.float16
    x = np.asarray(inputs["x"]).astype(np.int64)
    nbr = np.asarray(inputs["neighbors"]).astype(np.int64).reshape(P_ALL, K)
    embed = np.asarray(inputs["embed"], dtype=np.float32)
    w = np.asarray(inputs["w"], dtype=np.float32).reshape(P_ALL, D, F)
    qw = np.asarray(inputs["qw"], dtype=np.float32).reshape(P_ALL, F, -1)
    kw = np.asarray(inputs["kw"], dtype=np.float32).reshape(P_ALL, F, -1)
    Wx = np.asarray(inputs["Wx"], dtype=np.float32).reshape(P_ALL, 3, D, F)
    Wn = np.asarray(inputs["Wn"], dtype=np.float32).reshape(P_ALL, 3, F, F)
    b = (
        np.asarray(inputs["bx"], dtype=np.float32)
        + np.asarray(inputs["bn"], dtype=np.float32)
    ).reshape(P_ALL, 3, F)

    h = embed[x]                                    # [N, D]
    hp = np.repeat(h, R, axis=0)                    # [P, D] center node per pair
    hn = h[nbr]                                     # [P, K, D]
    A = w @ kw @ qw.transpose(0, 2, 1) @ w.transpose(0, 2, 1)   # [P, D, D]
    WnW0 = w @ Wn[:, 0]
    WnW1 = w @ Wn[:, 1]

    def stack2(M):
        # [PPC, D, X] -> [NT, 128, U2*X]: rows r*64+d, cols u*X+j
        X = M.shape[2]
        return (
            M.reshape(NT, U2, 2, D, X)
            .transpose(0, 2, 3, 1, 4)
            .reshape(NT, 128, U2 * X)
        )

    in_maps = []
    for c in range(NCORES):
        s = slice(c * PPC, (c + 1) * PPC)
        A_c, hn_c, w_c = A[s], hn[s], w[s]
        Wx_c, Wn2_c = Wx[s], Wn[s, 2]
        W0_c, W1_c = WnW0[s], WnW1[s]
        b_c, hp_c = b[s], hp[s]

        SA = stack2(A_c.transpose(0, 2, 1))                     # A^T blocks
        SC = stack2(hn_c.transpose(0, 2, 1))                    # hn^T blocks
        SD = (
            hn_c.reshape(NT, U4, 4, K, D)
            .transpose(0, 2, 3, 1, 4)
            .reshape(NT, 128, U4 * D)
        )
        RZblk = np.concatenate(
            [
                np.concatenate([Wx_c[:, 0], W0_c], axis=1),   # [PPC, 128, F]
                np.concatenate([Wx_c[:, 1], W1_c], axis=1),
            ],
            axis=2,
        )                                                      # [PPC, 128, 2F]
        SRZ = (
            RZblk.reshape(NT, TILE, 128, 2 * F)
            .transpose(0, 2, 1, 3)
            .reshape(NT, 128, TILE * 2 * F)
        )
        SW = stack2(w_c)
        SHp = np.concatenate([Wx_c[:, 2], Wn2_c], axis=1)       # [PPC, 128, F]
        SH = (
            SHp.reshape(NT, U2, 2, 128, F)
            .transpose(0, 3, 1, 2, 4)
            .reshape(NT, 128, U2 * 2 * F)
        )
        hp_t = hp_c.reshape(NT, TILE, D)
        Z = np.zeros((NT, TILE, 128), np.float32)
        Z[:, 0::2, 0:D] = hp_t[:, 0::2]
        Z[:, 1::2, D:128] = hp_t[:, 1::2]
        HD = Z.transpose(0, 2, 1)
        Hb = hp_t.transpose(0, 2, 1)
        B01 = (
            np.concatenate([b_c[:, 0], b_c[:, 1]], axis=1)
            .reshape(NT, TILE, 128)
            .transpose(0, 2, 1)
        )
        b2t = b_c[:, 2].reshape(NT, TILE, F)
        Z2 = np.zeros((NT, TILE, 128), np.float32)
        Z2[:, 0::2, 0:F] = b2t[:, 0::2]
        Z2[:, 1::2, F:128] = b2t[:, 1::2]
        B2D = Z2.transpose(0, 2, 1)
        BB = np.concatenate([B01, B2D], axis=2)

        m = {
            "SA": np.ascontiguousarray(SA).astype(f16),
            "SC": np.ascontiguousarray(SC).astype(f16),
            "SD": np.ascontiguousarray(SD).astype(f16),
            "SRZ": np.ascontiguousarray(SRZ).astype(f16),
            "SW": np.ascontiguousarray(SW).astype(f16),
            "SH": np.ascontiguousarray(SH).astype(f16),
            "HD": np.ascontiguousarray(HD).astype(f16),
            "Hb": np.ascontiguousarray(Hb).astype(f16),
            "BB": np.ascontiguousarray(BB),
        }
        in_maps.append(m)
    return in_maps


def kernel(**inputs):
    from concourse.bass_utils import run_bass_kernel_spmd

    if "nc" not in _cache:
        _cache["nc"] = _build()
    in_maps = _prep(inputs)
    res = run_bass_kernel_spmd(_cache["nc"], in_maps, list(range(NCORES)))
    outs = []
    for c in range(NCORES):
        o = res.results[c]["out"]                   # [NT, F, U2]
        outs.append(o.transpose(0, 2, 1).reshape(NPC, F))
    return np.concatenate(outs, axis=0).astype(np.float32)
